# revision 8
# baseline (speedup 1.0000x reference)
"""MoE layer (top-2 routing, 8 experts) on 8 Trainium2 NeuronCores.

Strategy (expert parallelism, per sharding hint):
  - Host computes the gate (logits -> top-k -> softmax) and routes tokens:
    expert e's tokens are gathered, padded to a common capacity C, and sent
    to core e.  This is the host-side equivalent of the "all-to-all tokens
    by expert assignment" step.
  - Core e runs the expert FFN for its tokens:
        yT = (gelu(x @ W1[e] + b1[e]) @ W2[e] + b2[e])^T
    entirely on-device in a transpose-free layout:
      mm1:  h^T[f,c] = sum_k W1_blk[k,f].T @ x^T[k,c]   (W1 stationary)
      gelu: ACT engine, exact (erf) Gelu, bias b1 fused
      mm2:  y^T[d,c] = sum_f W2_blk[f,d].T @ h^T[f,c]   (W2 stationary)
    Activations/weights are bf16 (full PE rate), accumulation fp32 in PSUM.
  - Both weight stacks (16 MB bf16) are DMA'd into SBUF once at kernel
    start and stay resident; token chunks stream through, with the next
    chunk's x prefetched during the current chunk's matmuls.  The kernel
    is PE-bound (~451 us at 2.4 GHz for C=2112), so all DMA hides behind
    the matmuls except the first weight tile.
  - Host scatters y back, scaled by the gate weights, and sums the top-k
    expert contributions per token.

Hardcoded problem shape: x [4, 2048, 1024], E=8 experts, D=1024, F=4096.
"""

import numpy as np
import ml_dtypes

import concourse.bass as bass
import concourse.mybir as mybir
import concourse.tile as tile
from concourse import bacc
from concourse.bass_utils import run_bass_kernel_spmd

D = 1024
F = 4096
E = 8
KD = D // 128   # 8 k-tiles over D
KF = F // 128   # 32 k-tiles over F
NT = 512        # token chunk (matmul moving free dim / PSUM bank)

_KERNEL_CACHE = {}


def _chunks(C):
    """Token chunks: one narrow head chunk, then equal widths (<= NT).

    The head chunk (256 cols) needs only 512 KB of x, so it starts as soon
    as the DMA rings spin up (~8.5 us) and its ~55 us of matmul work covers
    the entire 16 MB weight-delivery window.  The rest are equal widths,
    all well above the ~128-cycle LDWEIGHTS shadow.
    """
    head = 256
    if C <= head + 256:
        head = 0
    rest = C - head
    nch = -(-rest // NT)
    base = (rest // nch) & ~7
    rem = rest - base * nch
    assert rem % 8 == 0
    widths = ([head] if head else []) + [base + 8 * (i < rem // 8) for i in range(nch)]
    out, c0 = [], 0
    for w in widths:
        out.append((c0, w))
        c0 += w
    assert c0 == C
    return out


def _build_kernel(C: int):
    """Per-core expert-FFN kernel for capacity C (multiple of 64)."""
    assert C % 64 == 0
    bf16 = mybir.dt.bfloat16
    f32 = mybir.dt.float32

    nc = bacc.Bacc("TRN2", target_bir_lowering=False, debug=False, num_devices=8)

    xT = nc.dram_tensor("xT", [KD, 128, C], bf16, kind="ExternalInput")
    w1 = nc.dram_tensor("w1", [KF, 128, KD * 128], bf16, kind="ExternalInput")
    w2 = nc.dram_tensor("w2", [KD, 128, KF * 128], bf16, kind="ExternalInput")
    b1 = nc.dram_tensor("b1", [128, KF], f32, kind="ExternalInput")
    b2 = nc.dram_tensor("b2", [128, KD], f32, kind="ExternalInput")
    yT = nc.dram_tensor("yT", [KD, 128, C], bf16, kind="ExternalOutput")

    chunks = _chunks(C)

    with tile.TileContext(nc) as tc:
        with (
            tc.tile_pool(name="const", bufs=1) as const,
            tc.tile_pool(name="xp", bufs=2) as xp,
            tc.tile_pool(name="hp", bufs=1) as hp,
            tc.tile_pool(name="yp", bufs=4) as yp,
            tc.tile_pool(name="psA", bufs=4, space="PSUM") as psA,
            tc.tile_pool(name="psB", bufs=4, space="PSUM") as psB,
        ):
            # x chunk prefetch (GpSimd queue, one chunk ahead of compute).
            # Chunk 0 instead rides at the HEAD of the Sync queue (the fast
            # ring the weights use), in k order, so mm1 can start on k=0 the
            # moment the ring spins up.
            x_tiles = {}

            def load_x(ci, eng):
                c0, w = chunks[ci]
                t = xp.tile([128, KD, NT], bf16)
                for k in range(KD):
                    eng.dma_start(t[:, k, :w], xT[k, :, c0 : c0 + w])
                x_tiles[ci] = t

            load_x(0, nc.sync)

            b1_sb = const.tile([128, KF], f32)
            nc.sync.dma_start(b1_sb[:], b1[:])
            b2_sb = const.tile([128, KD], f32)
            nc.sync.dma_start(b2_sb[:], b2[:])

            # Resident weights, all on the Sync queue so the stream that the
            # PE is about to consume gets full DMA bandwidth.  Interleaved in
            # need-order: mm1 eats one w1 f-tile per ~1.4 us, mm2 needs its
            # first w2 d-tile only ~45 us in, so 4:1 w1:w2 keeps both ahead.
            w1_sb = const.tile([128, KF, KD * 128], bf16)
            w2_sb = const.tile([128, KD, KF * 128], bf16)
            for d in range(KD):
                for f in range(4 * d, 4 * d + 4):
                    nc.sync.dma_start(w1_sb[:, f, :], w1[f])
                nc.sync.dma_start(w2_sb[:, d, :], w2[d])

            for ci, (c0, w) in enumerate(chunks):
                if ci + 1 < len(chunks):
                    load_x(ci + 1, nc.gpsimd)
                x_t = x_tiles.pop(ci)

                h_t = hp.tile([128, KF, NT], bf16)
                for f in range(KF):
                    ps = psA.tile([128, NT], f32)
                    for k in range(KD):
                        nc.tensor.matmul(
                            ps[:, :w],
                            w1_sb[:, f, k * 128 : (k + 1) * 128],
                            x_t[:, k, :w],
                            start=(k == 0),
                            stop=(k == KD - 1),
                        )
                    nc.scalar.activation(
                        h_t[:, f, :w],
                        ps[:, :w],
                        mybir.ActivationFunctionType.Gelu,
                        bias=b1_sb[:, f : f + 1],
                    )

                for d in range(KD):
                    ps2 = psB.tile([128, NT], f32)
                    for k2 in range(KF):
                        nc.tensor.matmul(
                            ps2[:, :w],
                            w2_sb[:, d, k2 * 128 : (k2 + 1) * 128],
                            h_t[:, k2, :w],
                            start=(k2 == 0),
                            stop=(k2 == KF - 1),
                        )
                    y_t = yp.tile([128, NT], bf16)
                    nc.vector.tensor_scalar_add(y_t[:, :w], ps2[:, :w], b2_sb[:, d : d + 1])
                    # y stores ride the Scalar queue so they never head-block
                    # the x prefetch (GpSimd) or the weight stream (Sync).
                    nc.scalar.dma_start(yT[d, :, c0 : c0 + w], y_t[:, :w])

    nc.compile()
    return nc


def _get_kernel(C: int):
    if C not in _KERNEL_CACHE:
        _KERNEL_CACHE[C] = _build_kernel(C)
    return _KERNEL_CACHE[C]


def _route(xf, Wg, bg, top_k):
    """Replicate the reference gate: logits -> top-k -> softmax."""
    logits = xf.astype(np.float32) @ Wg.astype(np.float32) + bg.astype(np.float32)
    # jax.lax.top_k: values sorted descending, ties broken by lower index.
    order = np.argsort(-logits, axis=1, kind="stable")
    sel = order[:, :top_k]                                      # [T, K]
    vals = np.take_along_axis(logits, sel, axis=1)              # [T, K]
    vmax = vals.max(axis=1, keepdims=True)
    ex = np.exp((vals - vmax).astype(np.float32))
    w = ex / ex.sum(axis=1, keepdims=True)                      # [T, K]
    return sel, w.astype(np.float32)


def _plan(x, Wg, bg, top_k):
    """Routing plan: token indices + gate weight per expert, capacity C."""
    B, S, _ = x.shape
    xf = np.ascontiguousarray(x.reshape(B * S, D).astype(np.float32))
    sel, w = _route(xf, Wg, bg, top_k)
    idx_list, gate_list = [], []
    for e in range(E):
        hit = (sel == e)                    # [T, K]
        tok = np.nonzero(hit.any(axis=1))[0]
        kslot = hit[tok].argmax(axis=1)
        idx_list.append(tok)
        gate_list.append(w[tok, kslot])
    C = max(128, int(-(-max(len(t) for t in idx_list) // 64)) * 64)
    return xf, idx_list, gate_list, C


def _pack_inputs(xf, idx_list, C, W1, b1, W2, b2):
    xf_bf = xf.astype(ml_dtypes.bfloat16)
    in_maps = []
    for e in range(E):
        tok = idx_list[e]
        xe = np.zeros((C, D), dtype=ml_dtypes.bfloat16)
        xe[: len(tok)] = xf_bf[tok]
        in_maps.append(
            {
                "xT": np.ascontiguousarray(xe.reshape(C, KD, 128).transpose(1, 2, 0)),
                "w1": np.ascontiguousarray(
                    W1[e].astype(ml_dtypes.bfloat16)
                    .reshape(KD, 128, KF, 128).transpose(2, 1, 0, 3)
                    .reshape(KF, 128, KD * 128)
                ),
                "w2": np.ascontiguousarray(
                    W2[e].astype(ml_dtypes.bfloat16)
                    .reshape(KF, 128, KD, 128).transpose(2, 1, 0, 3)
                    .reshape(KD, 128, KF * 128)
                ),
                "b1": np.ascontiguousarray(b1[e].reshape(KF, 128).T.astype(np.float32)),
                "b2": np.ascontiguousarray(b2[e].reshape(KD, 128).T.astype(np.float32)),
            }
        )
    return in_maps


def _combine(results, idx_list, gate_list, C, T):
    out = np.zeros((T, D), dtype=np.float32)
    for e in range(E):
        tok = idx_list[e]
        if len(tok) == 0:
            continue
        y_pack = results[e]["yT"]                           # [KD, 128, C] bf16
        ye = y_pack.astype(np.float32).transpose(2, 0, 1).reshape(C, D)[: len(tok)]
        out[tok] += gate_list[e][:, None] * ye
    return out


def kernel(x, W1, b1, W2, b2, Wg, bg, top_k):
    x = np.asarray(x)
    W1 = np.asarray(W1, dtype=np.float32)
    b1 = np.asarray(b1, dtype=np.float32)
    W2 = np.asarray(W2, dtype=np.float32)
    b2 = np.asarray(b2, dtype=np.float32)
    Wg = np.asarray(Wg, dtype=np.float32)
    bg = np.asarray(bg, dtype=np.float32)
    top_k = int(np.asarray(top_k))

    B, S, Din = x.shape
    xf, idx_list, gate_list, C = _plan(x, Wg, bg, top_k)
    nc = _get_kernel(C)
    in_maps = _pack_inputs(xf, idx_list, C, W1, b1, W2, b2)
    res = run_bass_kernel_spmd(nc, in_maps, list(range(E)))
    out = _combine(res.results, idx_list, gate_list, C, B * S)
    return out.reshape(B, S, Din).astype(np.float32)
